# revision 44
# baseline (speedup 1.0000x reference)
"""DifferentialAttention Trainium2 kernel (8-core SPMD).

Sharding: core c = 4*b + g  (b in {0,1} batch, g in {0..3} kv-head group).
Each core computes attention for 4 q-heads / 1 kv-head of one batch element
and a partial W_O product over its heads' channels; the host sums the 4
partials per batch element.

Layout strategy (per core):
  - host passes x[b] TRANSPOSED (xT [D, S]) so all projections run with the
    weight chunk as the stationary matmul operand and xT as the moving one,
    producing q^T / k^T / v^T in [channel, token] layout directly.
  - RoPE channel de-interleave is folded into the Wq / Wk column permutation
    on the host (dot products are invariant to a shared q/k permutation);
    the softmax scale 1/sqrt(HD) is folded into Wq.
  - scores are computed transposed (s^T [kpos, q]) which makes both matmul
    operands natural-layout; exp needs no row max (|scores| <~ 10); the
    softmax denominator comes from a ones-vector matmul accumulated in PSUM
    alongside the PV matmul.
  - per-head LayerNorm statistics over the channel (partition) dim come from
    a (1/128)-vector matmul; rstd = exp(-0.5*ln(var+eps)) keeps everything
    in the single `natural_log_exp_and_others` ACT table set.
"""

import numpy as np
from contextlib import ExitStack

import concourse.bass as bass
import concourse.tile as tile
from concourse import mybir
from concourse.tile import TileContext
from concourse.masks import make_identity
from concourse.bass_utils import run_bass_kernel_spmd

F32 = mybir.dt.float32
F32R = mybir.dt.float32r
BF16 = mybir.dt.bfloat16
NPBF16 = mybir.dt.np(BF16)
AF = mybir.ActivationFunctionType
ALU = mybir.AluOpType

B = 2
S = 2048
D = 2048
H = 16
HKV = 4
HD = 64
NH = 4            # heads per core
CHQ = 128 * NH    # q channels per core (512)
LAYER_IDX = 12
EPS = 1e-5
NEG_THRESH = -1e8

NT = S // 128     # 16 token tiles of 128
NCT = S // 512    # 4 token chunks of 512
NDJ = D // 128    # 16 contraction chunks of 128


def split_multiwaits(nc):
    """walrus on this toolchain accepts at most ONE sem-wait per instruction;
    hoist extra waits onto NoOps inserted just before the offender."""
    n_fixed = 0
    for f in nc.m.functions:
        for bb in f.blocks:
            i = 0
            insts = bb.instructions
            while i < len(insts):
                inst = insts[i]
                si = inst.sync_info
                if si is not None and si.on_wait is not None and len(si.on_wait) > 1:
                    extra = list(si.on_wait[:-1])
                    keep = [si.on_wait[-1]]
                    for w in extra:
                        nop = mybir.InstNoOp(
                            name=f"I-waitfix-{nc.next_id()}", ins=[], outs=[]
                        )
                        nop.engine = inst.engine
                        nop.sync_info = mybir.SyncInfo(on_wait=[w], on_update=[])
                        nc.register_instruction(nop)
                        insts.insert(i, nop)
                        i += 1
                        n_fixed += 1
                    si.on_wait = keep
                i += 1
    return n_fixed


def declare_io(nc):
    xT = nc.dram_tensor("xT", [D, S], BF16, kind="ExternalInput")
    wq = nc.dram_tensor("wq", [D, CHQ], BF16, kind="ExternalInput")
    wk = nc.dram_tensor("wk", [D, 128], BF16, kind="ExternalInput")
    wv = nc.dram_tensor("wv", [D, 128], BF16, kind="ExternalInput")
    cc = nc.dram_tensor("cc", [128, S], BF16, kind="ExternalInput")  # cos rows x4
    sp = nc.dram_tensor("sp", [128, S], BF16, kind="ExternalInput")  # [+sin,-sin]x2
    diagmT = nc.dram_tensor("diagmT", [NT, 128, 128], BF16, kind="ExternalInput")
    lam = nc.dram_tensor("lam", [128, 1], F32, kind="ExternalInput")
    lnw = nc.dram_tensor("lnw", [128, NH], F32, kind="ExternalInput")
    wo = nc.dram_tensor("wo", [CHQ, D], BF16, kind="ExternalInput")
    vecs = nc.dram_tensor("vecs", [128, 2], F32R, kind="ExternalInput")
    mats = nc.dram_tensor("mats", [128, 256], BF16, kind="ExternalInput")
    matsf = nc.dram_tensor("matsf", [128, 128], F32R, kind="ExternalInput")
    swm = nc.dram_tensor("swm", [128, 128], BF16, kind="ExternalInput")
    out = nc.dram_tensor("out", [S, D], F32, kind="ExternalOutput")
    return (xT, wq, wk, wv, cc, sp, diagmT, lam, lnw, wo, vecs, mats, matsf,
            swm, out)


def build_nc(daug=False, lam_val=0.0):
    nc = bass.Bass()
    (xT, wq, wk, wv, cc, sp, diagmT, lam, lnw, wo, vecs, mats, matsf, swm,
     out) = declare_io(nc)

    with ExitStack() as ctx:
        tc = ctx.enter_context(TileContext(nc))
        _body(ctx, tc, nc, xT, wq, wk, wv, cc, sp, diagmT, lam, lnw, wo, vecs,
              mats, matsf, swm, out, daug, lam_val)

    split_multiwaits(nc)
    return nc


def _body(ctx, tc, nc, xT, wq, wk, wv, cc, sp, diagmT, lam, lnw, wo, vecs, mats,
          matsf, swm, out, daug=False, lam_val=0.0):
    f32r = lambda ap: ap  # tiles are natively F32R now

    consts = ctx.enter_context(tc.tile_pool(name="consts", bufs=1))
    attn_res = ctx.enter_context(tc.tile_pool(name="attn_res", bufs=1))

    # ---- whole-kernel residents -------------------------------------------
    diag_sb = consts.tile([128, NT, 128], BF16)
    nc.sync.dma_start(out=diag_sb, in_=diagmT[:, :, :].rearrange("n p c -> p n c"))
    lam_sb = consts.tile([128, 1], F32)
    nc.sync.dma_start(out=lam_sb, in_=lam[:, :])
    lnw_sb = consts.tile([128, NH], F32)
    nc.sync.dma_start(out=lnw_sb, in_=lnw[:, :])
    vecs_sb = consts.tile([128, 2], F32R)
    nc.sync.dma_start(out=vecs_sb, in_=vecs[:, :])
    ones_sb = vecs_sb[:, 0:1]
    inv_sb = vecs_sb[:, 1:2]
    mats_sb = consts.tile([128, 256], BF16)
    nc.sync.dma_start(out=mats_sb, in_=mats[:, :])
    onesm = mats_sb[:, 0:128]
    invm = mats_sb[:, 128:256]
    onesf = consts.tile([128, 128], F32R)
    nc.sync.dma_start(out=onesf, in_=matsf[:, :])
    swm_sb = consts.tile([128, 128], BF16)
    nc.sync.dma_start(out=swm_sb, in_=swm[:, :])
    ident = consts.tile([128, 128], BF16)
    make_identity(nc, ident)

    qrot = [attn_res.tile([128, S], BF16, tag=f"qrot{h}", name=f"qrot{h}") for h in range(NH)]
    krot = attn_res.tile([128, S], BF16)
    v_sb = attn_res.tile([128, NT, 128], BF16)

    # ---- phase 1: projections + rope + v transpose ------------------------
    with (
        tc.tile_pool(name="wq_pool", bufs=1) as wq_pool,
        tc.tile_pool(name="wkv_pool", bufs=1) as wkv_pool,
        tc.tile_pool(name="xt_pool", bufs=2) as xt_pool,
        tc.tile_pool(name="pcopy", bufs=2) as pcopy,
        tc.tile_pool(name="rtmp", bufs=2) as rtmp,
        tc.tile_pool(name="ps_proj", bufs=2, space="PSUM") as ps_proj,
        tc.tile_pool(name="ps_vt", bufs=1, space="PSUM") as ps_vt,
        tc.tile_pool(name="ps_rope", bufs=1, space="PSUM") as ps_rope,
    ):
        cc_sb = wkv_pool.tile([128, S], BF16)
        nc.sync.dma_start(out=cc_sb, in_=cc[:, :])
        sp_sb = wkv_pool.tile([128, S], BF16)
        nc.sync.dma_start(out=sp_sb, in_=sp[:, :])
        wq_sb = wq_pool.tile([128, NDJ, CHQ], BF16)
        nc.sync.dma_start(out=wq_sb, in_=wq[:, :].rearrange("(n p) c -> p n c", p=128))
        wk_sb = wkv_pool.tile([128, NDJ, 128], BF16)
        nc.sync.dma_start(out=wk_sb, in_=wk[:, :].rearrange("(n p) c -> p n c", p=128))
        wv_sb = wkv_pool.tile([128, NDJ, 128], BF16)
        nc.sync.dma_start(out=wv_sb, in_=wv[:, :].rearrange("(n p) c -> p n c", p=128))

        for ct in range(NCT):
            tsl = slice(512 * ct, 512 * ct + 512)
            xt = xt_pool.tile([128, NDJ, 512], BF16, tag="xt")
            for dj in range(NDJ):
                nc.sync.dma_start(
                    out=xt[:, dj, :], in_=xT[128 * dj : 128 * dj + 128, tsl]
                )

            def rope_to(dst, src):
                # src [128, 512] one head in [f][A32|B32] channel layout
                # (SBUF).  cc_sb rows: cos replicated per 32-block; sp_sb
                # rows: [-sin, +sin, -sin, +sin] per 32-block.  The A<->B
                # 32-row swap runs on the PE (swap-permutation stationary)
                # so the DVE does 3 full-width ops instead of 7 partial ones
                # (DVE cost is per free-dim column regardless of rows).
                ps_r = ps_rope.tile([128, 512], F32, tag="pr")
                nc.tensor.matmul(ps_r, swm_sb, src, start=True, stop=True)
                t1 = rtmp.tile([128, 512], BF16, tag="t1")
                nc.vector.tensor_mul(t1, src, cc_sb[:, tsl])
                t2 = rtmp.tile([128, 512], BF16, tag="t2")
                nc.vector.tensor_mul(t2, ps_r, sp_sb[:, tsl])
                nc.vector.tensor_add(dst[:, tsl], t1, t2)

            # q projection per head chunk
            for hc in range(NH):
                ps_q = ps_proj.tile([128, 512], F32, tag="psq")
                for dj in range(NDJ):
                    nc.tensor.matmul(
                        ps_q,
                        f32r(wq_sb[:, dj, 128 * hc : 128 * hc + 128]),
                        f32r(xt[:, dj, :]),
                        start=(dj == 0),
                        stop=(dj == NDJ - 1),
                    )
                qc = pcopy.tile([128, 512], BF16, tag="qc")
                nc.vector.tensor_copy(qc, ps_q)
                rope_to(qrot[hc], qc)

            # k projection
            ps_k = ps_proj.tile([128, 512], F32, tag="psk")
            for dj in range(NDJ):
                nc.tensor.matmul(
                    ps_k,
                    f32r(wk_sb[:, dj, :]),
                    f32r(xt[:, dj, :]),
                    start=(dj == 0),
                    stop=(dj == NDJ - 1),
                )
            kc = pcopy.tile([128, 512], BF16, tag="kc")
            nc.vector.tensor_copy(kc, ps_k)
            rope_to(krot, kc)

            # v projection (v^T) then PE-transpose to straight [tok, ch]
            ps_v = ps_proj.tile([128, 512], F32, tag="psv")
            for dj in range(NDJ):
                nc.tensor.matmul(
                    ps_v,
                    f32r(wv_sb[:, dj, :]),
                    f32r(xt[:, dj, :]),
                    start=(dj == 0),
                    stop=(dj == NDJ - 1),
                )
            vc = pcopy.tile([128, 512], BF16, tag="vc")
            nc.vector.tensor_copy(vc, ps_v)
            for u in range(4):
                tt = 4 * ct + u
                ps_t = ps_vt.tile([128, 128], BF16, tag="pvt")
                nc.tensor.transpose(ps_t, vc[:, 128 * u : 128 * u + 128], ident)
                nc.vector.tensor_copy(v_sb[:, tt, :], ps_t)
                if daug:
                    # v' = v + (1 - rowsum(v))/128 so every token's channel
                    # sum is 1: softmax denominators then fall out of the PV
                    # accumulator as channel sums, and the extra channel-
                    # constant shift cancels inside per-head LN.  With bf16 v
                    # the spread pass leaves rowsum error ~0.2 (128 roundings)
                    # so two greedy single-channel passes push the residual to
                    # the ulp floor (~4e-3).
                    sv = rtmp.tile([128, 1], F32, tag="sv")
                    nc.vector.reduce_sum(
                        out=sv, in_=v_sb[:, tt, :], axis=mybir.AxisListType.X
                    )
                    cval = rtmp.tile([128, 1], F32, tag="cval")
                    nc.vector.tensor_scalar(
                        out=cval, in0=sv, scalar1=-1.0 / 128.0,
                        scalar2=1.0 / 128.0, op0=ALU.mult, op1=ALU.add,
                    )
                    nc.vector.tensor_scalar(
                        out=v_sb[:, tt, :], in0=v_sb[:, tt, :],
                        scalar1=cval, scalar2=None, op0=ALU.add,
                    )
                    for fch in range(2):
                        sv2 = rtmp.tile([128, 1], F32, tag="sv")
                        nc.vector.reduce_sum(
                            out=sv2, in_=v_sb[:, tt, :],
                            axis=mybir.AxisListType.X,
                        )
                        r = rtmp.tile([128, 1], F32, tag="cval")
                        nc.vector.tensor_scalar(
                            out=r, in0=sv2, scalar1=-1.0, scalar2=1.0,
                            op0=ALU.mult, op1=ALU.add,
                        )
                        nc.vector.tensor_add(
                            v_sb[:, tt, fch : fch + 1],
                            v_sb[:, tt, fch : fch + 1], r,
                        )

    # ---- phase 2: attention ------------------------------------------------
    with tc.tile_pool(name="y_pool", bufs=1) as y_pool:
        y_all = y_pool.tile([128, NH, S], BF16)

        _attention_v2(tc, nc, qrot, krot, v_sb, diag_sb, lam_sb, lnw_sb,
                      onesm, invm, onesf, y_all, wo, out, daug)


def _attention(tc, nc, f32r, qrot, krot, v_sb, diag_sb, lam_sb, lnw_sb,
               ones_sb, inv_sb, y_all):
    with (
        tc.tile_pool(name="pP", bufs=12) as pP,
        tc.tile_pool(name="pbc", bufs=6) as pbc,
        tc.tile_pool(name="pcb", bufs=2) as pcb,
        tc.tile_pool(name="psm", bufs=8) as psm,
        tc.tile_pool(name="pdram", bufs=6, space="DRAM") as pdram,
        tc.tile_pool(name="ps_A", bufs=3, space="PSUM") as ps_A,
        tc.tile_pool(name="ps_sc", bufs=3, space="PSUM") as ps_sc,
        tc.tile_pool(name="ps_small", bufs=3, space="PSUM") as ps_small,
    ):
        def bcast(src, nm):
            # broadcast a [1, 512] row to [128, 512] via a DRAM bounce
            # (SBUF source APs may not have a zero partition step; DRAM may)
            scr = pdram.tile([1, 512], F32, tag="scr", name=f"scr_{nm}")
            nc.sync.dma_start(out=scr, in_=src)
            dst = pbc.tile([128, 512], F32, tag="bc", name=f"bc_{nm}")
            nc.gpsimd.dma_start(out=dst, in_=scr.to_broadcast([128, 512]))
            return dst

        for h in range(NH):
            for Q in range(NCT):
                q0 = 512 * Q
                jmax = 4 * Q + 3
                psA = [ps_A.tile([128, 512], F32, tag="A", name=f"psA{h}_{Q}_{s}") for s in range(2)]
                psD = [ps_small.tile([1, 512], F32, tag="sm", name=f"psD{h}_{Q}_{s}") for s in range(2)]
                for j in range(jmax + 1):
                    off = max(0, 128 * j - q0)
                    w = 512 - off
                    for s_i in range(2):
                        ps_s = ps_sc.tile([128, 512], F32, tag="s")
                        nc.tensor.matmul(
                            ps_s[:, off:],
                            f32r(krot[64 * s_i : 64 * s_i + 64, 128 * j : 128 * j + 128]),
                            f32r(qrot[h][64 * s_i : 64 * s_i + 64, q0 + off : q0 + 512]),
                            start=True,
                            stop=True,
                        )
                        if j >= 4 * Q:
                            nc.vector.tensor_add(
                                ps_s[:, off : off + 128],
                                ps_s[:, off : off + 128],
                                diag_sb[:, j, :],
                            )
                        P = pP.tile([128, 512], F32R, tag="P")
                        nc.scalar.activation(P[:, off:], ps_s[:, off:], AF.Exp)
                        nc.tensor.matmul(
                            psA[s_i][:, off:],
                            f32r(v_sb[:, j, :]),
                            f32r(P[:, off:]),
                            start=(j == 0),
                            stop=(j == jmax),
                        )
                        nc.tensor.matmul(
                            psD[s_i][:, off:],
                            f32r(ones_sb),
                            f32r(P[:, off:]),
                            start=(j == 0),
                            stop=(j == jmax),
                        )

                # denominators -> broadcast reciprocals
                rd = []
                for s_i in range(2):
                    r = psm.tile([1, 512], F32, tag="sm1")
                    nc.vector.reciprocal(r, psD[s_i])
                    rd.append(bcast(r, f"rd{h}_{Q}_{s_i}"))

                ych = y_all[:, h, q0 : q0 + 512]
                t2 = pcb.tile([128, 512], F32, tag="t2c")
                nc.vector.scalar_tensor_tensor(
                    out=t2, in0=psA[1], scalar=lam_sb, in1=rd[1],
                    op0=ALU.mult, op1=ALU.mult,
                )
                t1 = pcb.tile([128, 512], F32, tag="t1c")
                nc.vector.tensor_mul(t1, psA[0], rd[0])
                nc.vector.tensor_sub(ych, t1, t2)

                # LN stats: mu = (1/128) ones . Y ; Esq = (1/128) ones . Y^2
                psMu = ps_small.tile([1, 512], F32, tag="sm")
                nc.tensor.matmul(psMu, f32r(inv_sb), f32r(ych), start=True, stop=True)
                y2 = pcb.tile([128, 512], F32R, tag="y2")
                nc.scalar.activation(y2, ych, AF.Square)
                psSq = ps_small.tile([1, 512], F32, tag="sm")
                nc.tensor.matmul(psSq, f32r(inv_sb), f32r(y2), start=True, stop=True)

                m2 = psm.tile([1, 512], F32, tag="sm1")
                nc.scalar.activation(m2, psMu, AF.Square)
                var = psm.tile([1, 512], F32, tag="sm1")
                nc.vector.scalar_tensor_tensor(
                    out=var, in0=psSq, scalar=float(EPS), in1=m2,
                    op0=ALU.add, op1=ALU.subtract,
                )
                lnv = psm.tile([1, 512], F32, tag="sm1")
                nc.scalar.activation(lnv, var, AF.Ln)
                rstd = psm.tile([1, 512], F32, tag="sm1")
                nc.scalar.activation(rstd, lnv, AF.Exp, scale=-0.5)
                mr = psm.tile([1, 512], F32, tag="sm1")
                nc.vector.tensor_mul(mr, psMu, rstd)

                rstdb = bcast(rstd, f"rstd{h}_{Q}")
                mrb = bcast(mr, f"mr{h}_{Q}")

                # Z = (Y*lnw)*rstdb - mrb*lnw   (in place into y_all)
                tZ = pcb.tile([128, 512], F32, tag="tz")
                nc.vector.scalar_tensor_tensor(
                    out=tZ, in0=ych, scalar=lnw_sb[:, h : h + 1], in1=rstdb,
                    op0=ALU.mult, op1=ALU.mult,
                )
                tU = pcb.tile([128, 512], F32, tag="tu")
                nc.vector.tensor_scalar_mul(tU, mrb, lnw_sb[:, h : h + 1])
                nc.vector.tensor_sub(ych, tZ, tU)



def _attention_v2(tc, nc, qrot, krot, v_sb, diag_sb, lam_sb, lnw_sb,
                  onesm, invm, onesf, y_all, wo, out, daug=False):
    """Attention + interleaved W_O.

    Q-chunk outer / head inner; once all four heads of a Q-chunk have been
    layer-normed into y_all, the W_O matmuls for its four token tiles are
    emitted so they fill TensorE gaps in the next chunk's attention.
    Denominator / LN-stat matmuls use replicated [128,128] stationaries so
    their PSUM outputs need no cross-partition broadcast.  Work tiles share
    one pool tag; PSUM: 4 accumulator banks + 3 score banks + 1 W_O bank.

    daug=True (requires channel-constant lambda + the phase-1 v
    augmentation): the per-j denominator matmuls are dropped; D is the
    channel sum of the finished A accumulator, computed with ONE
    replicated-ones matmul over an f32 SBUF copy of psA (f32, not bf16 —
    the channel sum cancels heavily, so bf16 rounding of A would put ~2%
    noise on D).
    """
    with (
        tc.tile_pool(name="pP", bufs=8 if daug else 12) as pP,
        tc.tile_pool(name="pwk", bufs=16) as pwk,
        tc.tile_pool(name="pac", bufs=2) as pac,
        tc.tile_pool(name="wo_pool", bufs=1) as wo_pool,
        tc.tile_pool(name="ps_acc", bufs=3 if daug else 4, space="PSUM") as ps_acc,
        tc.tile_pool(name="ps_sc", bufs=2 if daug else 3, space="PSUM") as ps_sc,
        tc.tile_pool(name="ps_wo", bufs=1, space="PSUM") as ps_wo,
    ):
        wo_sb = wo_pool.tile([128, NH, D], BF16)
        nc.sync.dma_start(out=wo_sb, in_=wo[:, :].rearrange("(n p) c -> p n c", p=128))

        def wk(nm):
            return pwk.tile([128, 512], F32, tag="wk", name=nm)

        # big-Q chunks are PE-heavy (long j-loops), small-Q chunks are
        # DVE-heavy (fixed combine/LN tail); order them so tails overlap
        # long j-loops.  W_O for a Q-chunk fires once its heads complete.
        for Q in (3, 0, 2, 1):
            q0 = 512 * Q
            jmax = 4 * Q + 3
            for h in range(NH):
                t1 = None
                ych = y_all[:, h, q0 : q0 + 512]
                if daug:
                    # fused j-loop: both split-halves share one [128,1024]
                    # score tile so exp runs as one wide activation (ACT
                    # per-instruction overhead is ~293 ns — halving the
                    # instruction count matters more than tile shape).
                    psA2 = [ps_acc.tile([128, 512], F32, tag="acc",
                                        name=f"psA{h}_{Q}_{s}")
                            for s in range(2)]
                    for j in range(jmax + 1):
                        off = max(0, 128 * j - q0)
                        ps_s = ps_sc.tile([128, 1024], F32, tag="s",
                                          name=f"s{h}_{Q}_{j}")
                        for s_i in range(2):
                            nc.tensor.matmul(
                                ps_s[:, 512 * s_i + off : 512 * s_i + 512],
                                krot[64 * s_i : 64 * s_i + 64,
                                     128 * j : 128 * j + 128],
                                qrot[h][64 * s_i : 64 * s_i + 64,
                                        q0 + off : q0 + 512],
                                start=True,
                                stop=True,
                            )
                        P = pP.tile([128, 1024], BF16, tag="P",
                                    name=f"P{h}_{Q}_{j}")
                        if off == 0:
                            nc.scalar.activation(P, ps_s, AF.Exp)
                        else:
                            nc.scalar.activation(
                                P[:, off:512], ps_s[:, off:512], AF.Exp)
                            nc.scalar.activation(
                                P[:, 512 + off :], ps_s[:, 512 + off :],
                                AF.Exp)
                        if j >= 4 * Q:
                            for s_i in range(2):
                                nc.vector.tensor_mul(
                                    P[:, 512 * s_i + off :
                                      512 * s_i + off + 128],
                                    P[:, 512 * s_i + off :
                                      512 * s_i + off + 128],
                                    diag_sb[:, j, :],
                                )
                        for s_i in range(2):
                            nc.tensor.matmul(
                                psA2[s_i][:, off:], v_sb[:, j, :],
                                P[:, 512 * s_i + off : 512 * s_i + 512],
                                start=(j == 0), stop=(j == jmax),
                            )
                    for s_i in range(2):
                        rd = wk(f"rd{h}_{Q}_{s_i}")
                        ac = pac.tile([128, 512], F32R, tag="ac",
                                      name=f"ac{h}_{Q}_{s_i}")
                        nc.vector.tensor_copy(ac, psA2[s_i])
                        psDs = ps_acc.tile([128, 512], F32, tag="acc",
                                           name=f"psDs{h}_{Q}_{s_i}")
                        nc.tensor.matmul(psDs, onesf, ac, start=True,
                                         stop=True)
                        # 1/D as exp(-ln(D)) on ACT: nc.vector.reciprocal is
                        # an 8-cycle/elem iterative divide (~4.3us per tile);
                        # Ln+Exp share the already-loaded table set and cost
                        # ~0.7us each.
                        lnD = wk(f"lnD{h}_{Q}_{s_i}")
                        nc.scalar.activation(lnD, psDs, AF.Ln)
                        nc.scalar.activation(rd, lnD, AF.Exp, scale=-1.0)
                        if s_i == 0:
                            t1 = wk(f"t1_{h}_{Q}")
                            nc.vector.tensor_mul(t1, ac, rd)
                        else:
                            t2 = wk(f"t2_{h}_{Q}")
                            nc.vector.scalar_tensor_tensor(
                                out=t2, in0=ac, scalar=lam_sb, in1=rd,
                                op0=ALU.mult, op1=ALU.mult,
                            )
                            nc.vector.tensor_sub(ych, t1, t2)
                else:
                  for s_i in range(2):
                    psA = ps_acc.tile([128, 512], F32, tag="acc",
                                      name=f"psA{h}_{Q}_{s_i}")
                    psD = ps_acc.tile([128, 512], F32, tag="acc",
                                      name=f"psD{h}_{Q}_{s_i}")
                    for j in range(jmax + 1):
                        off = max(0, 128 * j - q0)
                        ps_s = ps_sc.tile([128, 512], F32, tag="s",
                                          name=f"s{h}_{Q}_{j}_{s_i}")
                        nc.tensor.matmul(
                            ps_s[:, off:],
                            krot[64 * s_i : 64 * s_i + 64,
                                 128 * j : 128 * j + 128],
                            qrot[h][64 * s_i : 64 * s_i + 64,
                                    q0 + off : q0 + 512],
                            start=True,
                            stop=True,
                        )
                        P = pP.tile([128, 512], BF16, tag="P",
                                    name=f"P{h}_{Q}_{j}_{s_i}")
                        nc.scalar.activation(P[:, off:], ps_s[:, off:], AF.Exp)
                        if j >= 4 * Q:
                            # 0/1 causal mask applied post-exp (SBUF, 2x mode,
                            # off the score-PSUM critical path); exp(s)*0 is
                            # bit-identical to exp(s - 1e9)
                            nc.vector.tensor_mul(
                                P[:, off : off + 128],
                                P[:, off : off + 128],
                                diag_sb[:, j, :],
                            )
                        nc.tensor.matmul(
                            psA[:, off:], v_sb[:, j, :], P[:, off:],
                            start=(j == 0), stop=(j == jmax),
                        )
                        nc.tensor.matmul(
                            psD[:, off:], onesm, P[:, off:],
                            start=(j == 0), stop=(j == jmax),
                        )
                    rd = wk(f"rd{h}_{Q}_{s_i}")
                    nc.vector.reciprocal(rd, psD)
                    if s_i == 0:
                        t1 = wk(f"t1_{h}_{Q}")
                        nc.vector.tensor_mul(t1, psA, rd)
                    else:
                        t2 = wk(f"t2_{h}_{Q}")
                        nc.vector.scalar_tensor_tensor(
                            out=t2, in0=psA, scalar=lam_sb, in1=rd,
                            op0=ALU.mult, op1=ALU.mult,
                        )
                        nc.vector.tensor_sub(ych, t1, t2)

                # LN for this (h, Q) chunk, replicated stats
                psMu = ps_acc.tile([128, 512], F32, tag="acc",
                                   name=f"psMu{h}_{Q}")
                nc.tensor.matmul(psMu, invm, ych, start=True, stop=True)
                musb = wk(f"musb{h}_{Q}")
                nc.vector.tensor_copy(musb, psMu)
                y2 = pwk.tile([128, 512], BF16, tag="y2b", name=f"y2_{h}_{Q}",
                              bufs=2)
                nc.vector.tensor_mul(y2, ych, ych)
                psSq = ps_acc.tile([128, 512], F32, tag="acc",
                                   name=f"psSq{h}_{Q}")
                nc.tensor.matmul(psSq, invm, y2, start=True, stop=True)
                m2 = wk(f"m2_{h}_{Q}")
                nc.vector.tensor_mul(m2, musb, musb)
                var = wk(f"var{h}_{Q}")
                nc.vector.scalar_tensor_tensor(
                    out=var, in0=psSq, scalar=float(EPS), in1=m2,
                    op0=ALU.add, op1=ALU.subtract,
                )
                lnv = wk(f"lnv{h}_{Q}")
                nc.scalar.activation(lnv, var, AF.Ln)
                rstd = wk(f"rstd{h}_{Q}")
                nc.scalar.activation(rstd, lnv, AF.Exp, scale=-0.5)
                tc1 = wk(f"tc1_{h}_{Q}")
                nc.vector.tensor_sub(tc1, ych, musb)
                nc.vector.scalar_tensor_tensor(
                    out=ych, in0=tc1, scalar=lnw_sb[:, h : h + 1], in1=rstd,
                    op0=ALU.mult, op1=ALU.mult,
                )

            # W_O for this Q-chunk's four token tiles.  For the final
            # chunk in the order there is no attention left to fill PE
            # gaps, so its groups also borrow the (now idle) score banks.
            for u in range(4):
                tt = 4 * Q + u
                for cci in range(4):
                    if Q == 1 and cci % 2 == 1:
                        if daug:
                            psO = ps_sc.tile([128, 1024], F32, tag="s",
                                             name=f"psO{tt}_{cci}")[:, 0:512]
                        else:
                            psO = ps_sc.tile([128, 512], F32, tag="s",
                                             name=f"psO{tt}_{cci}")
                    else:
                        psO = ps_wo.tile([128, 512], F32, tag="O",
                                         name=f"psO{tt}_{cci}")
                    for h in range(NH):
                        nc.tensor.matmul(
                            psO,
                            y_all[:, h, 128 * tt : 128 * tt + 128],
                            wo_sb[:, h, 512 * cci : 512 * cci + 512],
                            start=(h == 0),
                            stop=(h == NH - 1),
                        )
                    osb = pwk.tile([128, 512], F32, tag="osb",
                                   name=f"osb{tt}_{cci}", bufs=5)
                    if daug:
                        # ACT carries the exp+recip load in the daug build;
                        # keep all output staging on the DVE.
                        nc.vector.tensor_copy(osb, psO)
                    elif (tt + cci) % 2 == 0:
                        nc.scalar.copy(osb, psO)
                    else:
                        nc.vector.tensor_copy(osb, psO)
                    nc.sync.dma_start(
                        out=out[128 * tt : 128 * tt + 128,
                                512 * cci : 512 * cci + 512],
                        in_=osb,
                    )


def _attention_daug(tc, nc, qrot, krot, v_sb, diag_sb, lam_sb, lnw_sb,
                    ones_sb, y_all, lam_val):
    """D-free attention: with v' columns summing to 1, D_s = sum_ch A'_s.
    Valid when the lambda vector is channel-constant (per-head LN then
    cancels the channel-constant shift t_s introduced by the augmentation)."""
    mu_const = (1.0 - lam_val) / 128.0
    with (
        tc.tile_pool(name="pP", bufs=3) as pP,
        tc.tile_pool(name="pbc", bufs=4) as pbc,
        tc.tile_pool(name="pcb", bufs=2) as pcb,
        tc.tile_pool(name="psm", bufs=4) as psm,
        tc.tile_pool(name="pstat", bufs=2) as pstat,
        tc.tile_pool(name="pdram", bufs=6, space="DRAM") as pdram,
        tc.tile_pool(name="ps_A", bufs=4, space="PSUM") as ps_A,
        tc.tile_pool(name="ps_sc", bufs=3, space="PSUM") as ps_sc,
    ):
        def bcast(src, nm, width):
            scr = pdram.tile([1, width], F32, tag="scr", name=f"scr_{nm}")
            nc.sync.dma_start(out=scr, in_=src)
            dst = pbc.tile([128, width], F32, tag=f"bc{width}", name=f"bc_{nm}",
                           bufs=4 if width == 512 else 2)
            nc.gpsimd.dma_start(out=dst, in_=scr.to_broadcast([128, width]))
            return dst

        for h in range(NH):
            sqsb = pstat.tile([1, S], F32, tag="sqrow", name=f"sq{h}", bufs=1)
            for Q in range(NCT):
                q0 = 512 * Q
                jmax = 4 * Q + 3
                psA = [ps_A.tile([128, 512], F32, tag="A", name=f"psA{h}_{Q}_{s}")
                       for s in range(2)]
                for j in range(jmax + 1):
                    off = max(0, 128 * j - q0)
                    ps_s = ps_sc.tile([128, 1024], F32, tag="s")
                    for s_i in range(2):
                        nc.tensor.matmul(
                            ps_s[:, 512 * s_i + off : 512 * s_i + 512],
                            krot[64 * s_i : 64 * s_i + 64,
                                 128 * j : 128 * j + 128],
                            qrot[h][64 * s_i : 64 * s_i + 64,
                                    q0 + off : q0 + 512],
                            start=True,
                            stop=True,
                        )
                        if j >= 4 * Q:
                            nc.vector.tensor_add(
                                ps_s[:, 512 * s_i + off : 512 * s_i + off + 128],
                                ps_s[:, 512 * s_i + off : 512 * s_i + off + 128],
                                diag_sb[:, j, :],
                            )
                    P = pP.tile([128, 1024], F32R, tag="P")
                    if off == 0:
                        nc.scalar.activation(P, ps_s, AF.Exp)
                    else:
                        nc.scalar.activation(
                            P[:, off:512], ps_s[:, off:512], AF.Exp)
                        nc.scalar.activation(
                            P[:, 512 + off :], ps_s[:, 512 + off :], AF.Exp)
                    for s_i in range(2):
                        nc.tensor.matmul(
                            psA[s_i][:, off:],
                            v_sb[:, j, :],
                            P[:, 512 * s_i + off : 512 * s_i + 512],
                            start=(j == 0),
                            stop=(j == jmax),
                        )

                # denominators = channel sums of A'; combine; LN row stats
                ac, rdb = [], []
                for s_i in range(2):
                    c = pcb.tile([128, 512], F32R, tag=f"ac{s_i}",
                                 name=f"ac{s_i}_{h}_{Q}", bufs=2)
                    nc.vector.tensor_copy(c, psA[s_i])
                    ac.append(c)
                for s_i in range(2):
                    Ssum = ps_A.tile([1, 512], F32, tag="A",
                                     name=f"S{s_i}_{h}_{Q}")
                    nc.tensor.matmul(Ssum, ones_sb, ac[s_i], start=True, stop=True)
                    r = psm.tile([1, 512], F32, tag="sm1", name=f"rd{s_i}_{h}_{Q}")
                    nc.vector.reciprocal(r, Ssum)
                    rdb.append(bcast(r, f"rd{s_i}_{h}_{Q}", 512))

                ych = y_all[:, h, q0 : q0 + 512]
                t2 = pcb.tile([128, 512], F32, tag="t2c")
                nc.vector.scalar_tensor_tensor(
                    out=t2, in0=ac[1], scalar=lam_sb, in1=rdb[1],
                    op0=ALU.mult, op1=ALU.mult,
                )
                t1 = pcb.tile([128, 512], F32, tag="t1c")
                nc.vector.tensor_mul(t1, ac[0], rdb[0])
                nc.vector.tensor_sub(ych, t1, t2)

                y2 = pcb.tile([128, 512], F32R, tag="y2")
                nc.vector.tensor_mul(y2, ych, ych)
                psSq = ps_A.tile([1, 512], F32, tag="A", name=f"psSq{h}_{Q}")
                nc.tensor.matmul(psSq, ones_sb, y2, start=True, stop=True)
                nc.vector.tensor_scalar(
                    out=sqsb[:, q0 : q0 + 512], in0=psSq,
                    scalar1=1.0 / 128.0, scalar2=None, op0=ALU.mult,
                )

            # per-head LN tail: var = E[y^2] - mu^2 (mu = (1-lam)/128 exact)
            var = psm.tile([1, S], F32, tag="var", name=f"var{h}", bufs=2)
            nc.vector.tensor_scalar(
                out=var, in0=sqsb,
                scalar1=float(EPS - mu_const * mu_const), scalar2=None,
                op0=ALU.add,
            )
            lnv = psm.tile([1, S], F32, tag="var", name=f"lnv{h}", bufs=2)
            nc.scalar.activation(lnv, var, AF.Ln)
            rstd = psm.tile([1, S], F32, tag="var", name=f"rstd{h}", bufs=2)
            nc.scalar.activation(rstd, lnv, AF.Exp, scale=-0.5)

            for Q in range(NCT):
                q0 = 512 * Q
                rb = bcast(rstd[:, q0 : q0 + 512], f"rstd{h}_{Q}", 512)
                ych = y_all[:, h, q0 : q0 + 512]
                tZ = pcb.tile([128, 512], F32, tag="tz", bufs=2)
                nc.vector.scalar_tensor_tensor(
                    out=tZ, in0=ych, scalar=lnw_sb[:, h : h + 1], in1=rb,
                    op0=ALU.mult, op1=ALU.mult,
                )
                tU = pcb.tile([128, 512], F32, tag="tu", bufs=2)
                nc.vector.tensor_scalar(
                    out=tU, in0=rb, scalar1=lnw_sb[:, h : h + 1],
                    scalar2=float(mu_const), op0=ALU.mult, op1=ALU.mult,
                )
                nc.vector.tensor_sub(ych, tZ, tU)

def _wo_phase(tc, nc, f32r, y_all, wo, out):
    with (
        tc.tile_pool(name="ps_wo", bufs=2, space="PSUM") as ps_wo,
        tc.tile_pool(name="ostage", bufs=4) as ostage,
        tc.tile_pool(name="wo_pool", bufs=1) as wo_pool,
    ):
        wo_sb = wo_pool.tile([128, NH, D], BF16)
        nc.sync.dma_start(out=wo_sb, in_=wo[:, :].rearrange("(n p) c -> p n c", p=128))
        for tt in range(NT):
            psO = ps_wo.tile([128, 2048], F32, tag="O")
            for cci in range(4):
                for h in range(NH):
                    nc.tensor.matmul(
                        psO[:, 512 * cci : 512 * cci + 512],
                        f32r(y_all[:, h, 128 * tt : 128 * tt + 128]),
                        f32r(wo_sb[:, h, 512 * cci : 512 * cci + 512]),
                        start=(h == 0),
                        stop=(h == NH - 1),
                    )
            osb = ostage.tile([128, 2048], F32, tag="osb")
            nc.scalar.copy(osb, psO)
            nc.sync.dma_start(
                out=out[128 * tt : 128 * tt + 128, :], in_=osb,
            )


# --------------------------------------------------------------------------
# host side
# --------------------------------------------------------------------------

_PERM = np.concatenate([np.arange(0, 64, 2), np.arange(1, 64, 2)])


def prep_core_inputs(x, mask, freq_cos, freq_sin, Wq, Wkv, Wo,
                     lambda_q1, lambda_k1, lambda_q2, lambda_k2, ln_weight,
                     b, g):
    lam_init = np.float32(0.8) - np.float32(0.6) * np.exp(
        np.float32(-0.3 * LAYER_IDX)
    )
    scale = np.float32(HD ** -0.5)

    Wq4 = Wq.reshape(D, H, 2, HD)[:, 4 * g : 4 * g + 4]
    Wq4 = Wq4[..., _PERM] * scale
    wq = np.ascontiguousarray(Wq4.reshape(D, CHQ)).astype(NPBF16)

    Wkv5 = Wkv.reshape(D, 2, HKV, 2 * HD)
    wk = Wkv5[:, 0, g].reshape(D, 2, HD)[..., _PERM].reshape(D, 2 * HD)
    wk = np.ascontiguousarray(wk).astype(NPBF16)
    wv = np.ascontiguousarray(Wkv5[:, 1, g]).astype(NPBF16)

    cosT = freq_cos.T.astype(np.float32)   # [32, S]
    sinT = freq_sin.T.astype(np.float32)
    cc = np.ascontiguousarray(np.vstack([cosT, cosT, cosT, cosT])).astype(NPBF16)
    sp = np.ascontiguousarray(np.vstack([-sinT, sinT, -sinT, sinT])).astype(NPBF16)

    m = mask[0, 0]
    diagmT = np.ascontiguousarray(
        np.stack(
            [(m[i * 128 : (i + 1) * 128, i * 128 : (i + 1) * 128].T
              > NEG_THRESH).astype(np.float32) for i in range(NT)]
        ),
    ).astype(NPBF16)

    lamvec = (
        np.exp(lambda_q1 * lambda_k1) - np.exp(lambda_q2 * lambda_k2) + lam_init
    ).astype(np.float32).reshape(128, 1)
    lnw = np.ascontiguousarray(
        (ln_weight[4 * g : 4 * g + 4] * (np.float32(1.0) - lam_init)).T,
        dtype=np.float32,
    )
    wo = np.ascontiguousarray(Wo[CHQ * g : CHQ * (g + 1)]).astype(NPBF16)
    xT = np.ascontiguousarray(x[b].T).astype(NPBF16)

    return {
        "xT": xT, "wq": wq, "wk": wk, "wv": wv, "cc": cc, "sp": sp,
        "diagmT": diagmT, "lam": lamvec, "lnw": lnw, "wo": wo,
        "vecs": np.ascontiguousarray(
            np.stack([np.ones(128), np.full(128, 1.0 / 128.0)], axis=1),
            dtype=np.float32),
        "mats": np.ascontiguousarray(np.concatenate(
            [np.ones((128, 128)), np.full((128, 128), 1.0 / 128.0)], axis=1)
        ).astype(NPBF16),
        "matsf": np.ones((128, 128), dtype=np.float32),
        "swm": _swap_matrix(),
    }


def _swap_matrix():
    # permutation swapping the A/B 32-row halves of each 64-row block
    sw = np.zeros((128, 128), dtype=np.float32)
    for f in range(2):
        for i in range(32):
            sw[64 * f + i, 64 * f + 32 + i] = 1.0
            sw[64 * f + 32 + i, 64 * f + i] = 1.0
    return sw.astype(NPBF16)


def check_mask_causal(mask):
    m = mask[0, 0]
    lower_ok = True
    upper_ok = True
    tril = np.tril_indices(S)
    if not np.all(m[tril] == 0.0):
        lower_ok = False
    triu = np.triu_indices(S, k=1)
    if not np.all(m[triu] <= NEG_THRESH):
        upper_ok = False
    return lower_ok and upper_ok


_NC_CACHE = {}


def _lam_vec(inputs):
    lam_init = np.float32(0.8) - np.float32(0.6) * np.exp(np.float32(-0.3 * LAYER_IDX))
    return (
        np.exp(inputs["lambda_q1"] * inputs["lambda_k1"])
        - np.exp(inputs["lambda_q2"] * inputs["lambda_k2"])
        + lam_init
    ).astype(np.float32)


def _get_nc(daug=False, lam_val=0.0):
    key = (bool(daug), float(lam_val) if daug else 0.0)
    if key not in _NC_CACHE:
        _NC_CACHE[key] = build_nc(daug, lam_val)
    return _NC_CACHE[key]


def kernel(**inputs) -> np.ndarray:
    inputs = {k: np.asarray(v) for k, v in inputs.items()}
    assert check_mask_causal(inputs["mask"]), (
        "kernel assumes a causal additive mask (0 lower / <=-1e8 strict upper); "
        "got something else"
    )
    # lambda channel-constant (true for the reference's all-ones lambda
    # vectors) enables the denominator-free daug path; anything else falls
    # back to explicit denominator accumulation.
    daug = bool(np.ptp(_lam_vec(inputs)) == 0.0)
    nc = _get_nc(daug, 0.0)
    in_maps = []
    for b in range(B):
        for g in range(HKV):
            in_maps.append(prep_core_inputs(
                inputs["x"], inputs["mask"], inputs["freq_cos"], inputs["freq_sin"],
                inputs["Wq"], inputs["Wkv"], inputs["Wo"],
                inputs["lambda_q1"], inputs["lambda_k1"],
                inputs["lambda_q2"], inputs["lambda_k2"], inputs["ln_weight"],
                b, g,
            ))
    res = run_bass_kernel_spmd(nc, in_maps, list(range(B * HKV)))
    parts = [r["out"] for r in res.results]
    out = np.stack([
        np.sum(np.stack(parts[0:HKV]), axis=0, dtype=np.float32),
        np.sum(np.stack(parts[HKV : 2 * HKV]), axis=0, dtype=np.float32),
    ])
    return out.astype(np.float32)



# revision 48
# speedup vs baseline: 1.0791x; 1.0791x over previous
"""DifferentialAttention Trainium2 kernel (8-core SPMD).

Sharding: core c = 4*b + g  (b in {0,1} batch, g in {0..3} kv-head group).
Each core computes attention for 4 q-heads / 1 kv-head of one batch element
and a partial W_O product over its heads' channels; the host sums the 4
partials per batch element.

Layout strategy (per core):
  - host passes x[b] TRANSPOSED (xT [D, S]) so all projections run with the
    weight chunk as the stationary matmul operand and xT as the moving one,
    producing q^T / k^T / v^T in [channel, token] layout directly.
  - RoPE channel de-interleave is folded into the Wq / Wk column permutation
    on the host (dot products are invariant to a shared q/k permutation);
    the softmax scale 1/sqrt(HD) is folded into Wq.
  - scores are computed transposed (s^T [kpos, q]) which makes both matmul
    operands natural-layout; exp needs no row max (|scores| <~ 10); the
    softmax denominator comes from a ones-vector matmul accumulated in PSUM
    alongside the PV matmul.
  - per-head LayerNorm statistics over the channel (partition) dim come from
    a (1/128)-vector matmul; rstd = exp(-0.5*ln(var+eps)) keeps everything
    in the single `natural_log_exp_and_others` ACT table set.
"""

import numpy as np
from contextlib import ExitStack

import concourse.bass as bass
import concourse.tile as tile
from concourse import mybir
from concourse.tile import TileContext
from concourse.masks import make_identity
from concourse.bass_utils import run_bass_kernel_spmd

F32 = mybir.dt.float32
F32R = mybir.dt.float32r
BF16 = mybir.dt.bfloat16
NPBF16 = mybir.dt.np(BF16)
AF = mybir.ActivationFunctionType
ALU = mybir.AluOpType

B = 2
S = 2048
D = 2048
H = 16
HKV = 4
HD = 64
NH = 4            # heads per core
CHQ = 128 * NH    # q channels per core (512)
LAYER_IDX = 12
EPS = 1e-5
NEG_THRESH = -1e8

NT = S // 128     # 16 token tiles of 128
NCT = S // 512    # 4 token chunks of 512
NDJ = D // 128    # 16 contraction chunks of 128


def split_multiwaits(nc):
    """walrus on this toolchain accepts at most ONE sem-wait per instruction;
    hoist extra waits onto NoOps inserted just before the offender."""
    n_fixed = 0
    for f in nc.m.functions:
        for bb in f.blocks:
            i = 0
            insts = bb.instructions
            while i < len(insts):
                inst = insts[i]
                si = inst.sync_info
                if si is not None and si.on_wait is not None and len(si.on_wait) > 1:
                    extra = list(si.on_wait[:-1])
                    keep = [si.on_wait[-1]]
                    for w in extra:
                        nop = mybir.InstNoOp(
                            name=f"I-waitfix-{nc.next_id()}", ins=[], outs=[]
                        )
                        nop.engine = inst.engine
                        nop.sync_info = mybir.SyncInfo(on_wait=[w], on_update=[])
                        nc.register_instruction(nop)
                        insts.insert(i, nop)
                        i += 1
                        n_fixed += 1
                    si.on_wait = keep
                i += 1
    return n_fixed


def declare_io(nc):
    xT = nc.dram_tensor("xT", [D, S], BF16, kind="ExternalInput")
    wq = nc.dram_tensor("wq", [D, CHQ], BF16, kind="ExternalInput")
    wk = nc.dram_tensor("wk", [D, 128], BF16, kind="ExternalInput")
    wv = nc.dram_tensor("wv", [D, 128], BF16, kind="ExternalInput")
    cc = nc.dram_tensor("cc", [128, S], BF16, kind="ExternalInput")  # cos rows x4
    sp = nc.dram_tensor("sp", [128, S], BF16, kind="ExternalInput")  # [+sin,-sin]x2
    diagmT = nc.dram_tensor("diagmT", [NT, 128, 128], BF16, kind="ExternalInput")
    lam = nc.dram_tensor("lam", [128, 1], F32, kind="ExternalInput")
    lnw = nc.dram_tensor("lnw", [128, NH], F32, kind="ExternalInput")
    wo = nc.dram_tensor("wo", [CHQ, D], BF16, kind="ExternalInput")
    vecs = nc.dram_tensor("vecs", [128, 2], F32R, kind="ExternalInput")
    mats = nc.dram_tensor("mats", [128, 256], BF16, kind="ExternalInput")
    matsf = nc.dram_tensor("matsf", [128, 128], F32R, kind="ExternalInput")
    swm = nc.dram_tensor("swm", [128, 128], BF16, kind="ExternalInput")
    out = nc.dram_tensor("out", [S, D], F32, kind="ExternalOutput")
    return (xT, wq, wk, wv, cc, sp, diagmT, lam, lnw, wo, vecs, mats, matsf,
            swm, out)


def build_nc(daug=False, lam_val=0.0):
    nc = bass.Bass()
    (xT, wq, wk, wv, cc, sp, diagmT, lam, lnw, wo, vecs, mats, matsf, swm,
     out) = declare_io(nc)

    with ExitStack() as ctx:
        tc = ctx.enter_context(TileContext(nc))
        _body(ctx, tc, nc, xT, wq, wk, wv, cc, sp, diagmT, lam, lnw, wo, vecs,
              mats, matsf, swm, out, daug, lam_val)

    split_multiwaits(nc)
    return nc


def _body(ctx, tc, nc, xT, wq, wk, wv, cc, sp, diagmT, lam, lnw, wo, vecs, mats,
          matsf, swm, out, daug=False, lam_val=0.0):
    f32r = lambda ap: ap  # tiles are natively F32R now

    consts = ctx.enter_context(tc.tile_pool(name="consts", bufs=1))
    attn_res = ctx.enter_context(tc.tile_pool(name="attn_res", bufs=1))

    # ---- whole-kernel residents -------------------------------------------
    diag_sb = consts.tile([128, NT, 128], BF16)
    nc.sync.dma_start(out=diag_sb, in_=diagmT[:, :, :].rearrange("n p c -> p n c"))
    lam_sb = consts.tile([128, 1], F32)
    nc.sync.dma_start(out=lam_sb, in_=lam[:, :])
    lnw_sb = consts.tile([128, NH], F32)
    nc.sync.dma_start(out=lnw_sb, in_=lnw[:, :])
    vecs_sb = consts.tile([128, 2], F32R)
    nc.sync.dma_start(out=vecs_sb, in_=vecs[:, :])
    ones_sb = vecs_sb[:, 0:1]
    inv_sb = vecs_sb[:, 1:2]
    mats_sb = consts.tile([128, 256], BF16)
    nc.sync.dma_start(out=mats_sb, in_=mats[:, :])
    onesm = mats_sb[:, 0:128]
    invm = mats_sb[:, 128:256]
    onesf = consts.tile([128, 128], F32R)
    nc.sync.dma_start(out=onesf, in_=matsf[:, :])
    swm_sb = consts.tile([128, 128], BF16)
    nc.sync.dma_start(out=swm_sb, in_=swm[:, :])
    ident = consts.tile([128, 128], BF16)
    make_identity(nc, ident)

    qrot = [attn_res.tile([128, S], BF16, tag=f"qrot{h}", name=f"qrot{h}") for h in range(NH)]
    krot = attn_res.tile([128, S], BF16)
    v_sb = attn_res.tile([128, NT, 128], BF16)

    # ---- phase 1: projections + rope + v transpose ------------------------
    with (
        tc.tile_pool(name="wq_pool", bufs=1) as wq_pool,
        tc.tile_pool(name="wkv_pool", bufs=1) as wkv_pool,
        tc.tile_pool(name="xt_pool", bufs=2) as xt_pool,
        tc.tile_pool(name="pcopy", bufs=2) as pcopy,
        tc.tile_pool(name="rtmp", bufs=2) as rtmp,
        tc.tile_pool(name="ps_proj", bufs=2, space="PSUM") as ps_proj,
        tc.tile_pool(name="ps_vt", bufs=1, space="PSUM") as ps_vt,
        tc.tile_pool(name="ps_rope", bufs=1, space="PSUM") as ps_rope,
    ):
        cc_sb = wkv_pool.tile([128, S], BF16)
        nc.sync.dma_start(out=cc_sb, in_=cc[:, :])
        sp_sb = wkv_pool.tile([128, S], BF16)
        nc.sync.dma_start(out=sp_sb, in_=sp[:, :])
        wq_sb = wq_pool.tile([128, NDJ, CHQ], BF16)
        nc.sync.dma_start(out=wq_sb, in_=wq[:, :].rearrange("(n p) c -> p n c", p=128))
        wk_sb = wkv_pool.tile([128, NDJ, 128], BF16)
        nc.sync.dma_start(out=wk_sb, in_=wk[:, :].rearrange("(n p) c -> p n c", p=128))
        wv_sb = wkv_pool.tile([128, NDJ, 128], BF16)
        nc.sync.dma_start(out=wv_sb, in_=wv[:, :].rearrange("(n p) c -> p n c", p=128))

        for ct in range(NCT):
            tsl = slice(512 * ct, 512 * ct + 512)
            xt = xt_pool.tile([128, NDJ, 512], BF16, tag="xt")
            for dj in range(NDJ):
                nc.sync.dma_start(
                    out=xt[:, dj, :], in_=xT[128 * dj : 128 * dj + 128, tsl]
                )

            def rope_to(dst, src):
                # src [128, 512] one head in [f][A32|B32] channel layout
                # (SBUF).  cc_sb rows: cos replicated per 32-block; sp_sb
                # rows: [-sin, +sin, -sin, +sin] per 32-block.  The A<->B
                # 32-row swap runs on the PE (swap-permutation stationary)
                # so the DVE does 3 full-width ops instead of 7 partial ones
                # (DVE cost is per free-dim column regardless of rows).
                ps_r = ps_rope.tile([128, 512], F32, tag="pr")
                nc.tensor.matmul(ps_r, swm_sb, src, start=True, stop=True)
                t1 = rtmp.tile([128, 512], BF16, tag="t1")
                nc.vector.tensor_mul(t1, src, cc_sb[:, tsl])
                t2 = rtmp.tile([128, 512], BF16, tag="t2")
                nc.vector.tensor_mul(t2, ps_r, sp_sb[:, tsl])
                nc.vector.tensor_add(dst[:, tsl], t1, t2)

            # q projection per head chunk
            for hc in range(NH):
                ps_q = ps_proj.tile([128, 512], F32, tag="psq")
                for dj in range(NDJ):
                    nc.tensor.matmul(
                        ps_q,
                        f32r(wq_sb[:, dj, 128 * hc : 128 * hc + 128]),
                        f32r(xt[:, dj, :]),
                        start=(dj == 0),
                        stop=(dj == NDJ - 1),
                    )
                qc = pcopy.tile([128, 512], BF16, tag="qc")
                nc.vector.tensor_copy(qc, ps_q)
                rope_to(qrot[hc], qc)

            # k projection
            ps_k = ps_proj.tile([128, 512], F32, tag="psk")
            for dj in range(NDJ):
                nc.tensor.matmul(
                    ps_k,
                    f32r(wk_sb[:, dj, :]),
                    f32r(xt[:, dj, :]),
                    start=(dj == 0),
                    stop=(dj == NDJ - 1),
                )
            kc = pcopy.tile([128, 512], BF16, tag="kc")
            nc.vector.tensor_copy(kc, ps_k)
            rope_to(krot, kc)

            # v projection (v^T) then PE-transpose to straight [tok, ch]
            ps_v = ps_proj.tile([128, 512], F32, tag="psv")
            for dj in range(NDJ):
                nc.tensor.matmul(
                    ps_v,
                    f32r(wv_sb[:, dj, :]),
                    f32r(xt[:, dj, :]),
                    start=(dj == 0),
                    stop=(dj == NDJ - 1),
                )
            vc = pcopy.tile([128, 512], BF16, tag="vc")
            nc.vector.tensor_copy(vc, ps_v)
            for u in range(4):
                tt = 4 * ct + u
                ps_t = ps_vt.tile([128, 128], BF16, tag="pvt")
                nc.tensor.transpose(ps_t, vc[:, 128 * u : 128 * u + 128], ident)
                nc.vector.tensor_copy(v_sb[:, tt, :], ps_t)
                if daug:
                    # v' = v + (1 - rowsum(v))/128 so every token's channel
                    # sum is 1: softmax denominators then fall out of the PV
                    # accumulator as channel sums, and the extra channel-
                    # constant shift cancels inside per-head LN.  With bf16 v
                    # the spread pass leaves rowsum error ~0.2 (128 roundings)
                    # so two greedy single-channel passes push the residual to
                    # the ulp floor (~4e-3).
                    sv = rtmp.tile([128, 1], F32, tag="sv")
                    nc.vector.reduce_sum(
                        out=sv, in_=v_sb[:, tt, :], axis=mybir.AxisListType.X
                    )
                    cval = rtmp.tile([128, 1], F32, tag="cval")
                    nc.vector.tensor_scalar(
                        out=cval, in0=sv, scalar1=-1.0 / 128.0,
                        scalar2=1.0 / 128.0, op0=ALU.mult, op1=ALU.add,
                    )
                    nc.vector.tensor_scalar(
                        out=v_sb[:, tt, :], in0=v_sb[:, tt, :],
                        scalar1=cval, scalar2=None, op0=ALU.add,
                    )
                    for fch in range(2):
                        sv2 = rtmp.tile([128, 1], F32, tag="sv")
                        nc.vector.reduce_sum(
                            out=sv2, in_=v_sb[:, tt, :],
                            axis=mybir.AxisListType.X,
                        )
                        r = rtmp.tile([128, 1], F32, tag="cval")
                        nc.vector.tensor_scalar(
                            out=r, in0=sv2, scalar1=-1.0, scalar2=1.0,
                            op0=ALU.mult, op1=ALU.add,
                        )
                        nc.vector.tensor_add(
                            v_sb[:, tt, fch : fch + 1],
                            v_sb[:, tt, fch : fch + 1], r,
                        )

    # ---- phase 2: attention ------------------------------------------------
    with tc.tile_pool(name="y_pool", bufs=1) as y_pool:
        y_all = y_pool.tile([128, NH, S], BF16)

        _attention_v2(tc, nc, qrot, krot, v_sb, diag_sb, lam_sb, lnw_sb,
                      onesm, invm, onesf, y_all, wo, out, daug, lam_val)


def _attention(tc, nc, f32r, qrot, krot, v_sb, diag_sb, lam_sb, lnw_sb,
               ones_sb, inv_sb, y_all):
    with (
        tc.tile_pool(name="pP", bufs=12) as pP,
        tc.tile_pool(name="pbc", bufs=6) as pbc,
        tc.tile_pool(name="pcb", bufs=2) as pcb,
        tc.tile_pool(name="psm", bufs=8) as psm,
        tc.tile_pool(name="pdram", bufs=6, space="DRAM") as pdram,
        tc.tile_pool(name="ps_A", bufs=3, space="PSUM") as ps_A,
        tc.tile_pool(name="ps_sc", bufs=3, space="PSUM") as ps_sc,
        tc.tile_pool(name="ps_small", bufs=3, space="PSUM") as ps_small,
    ):
        def bcast(src, nm):
            # broadcast a [1, 512] row to [128, 512] via a DRAM bounce
            # (SBUF source APs may not have a zero partition step; DRAM may)
            scr = pdram.tile([1, 512], F32, tag="scr", name=f"scr_{nm}")
            nc.sync.dma_start(out=scr, in_=src)
            dst = pbc.tile([128, 512], F32, tag="bc", name=f"bc_{nm}")
            nc.gpsimd.dma_start(out=dst, in_=scr.to_broadcast([128, 512]))
            return dst

        for h in range(NH):
            for Q in range(NCT):
                q0 = 512 * Q
                jmax = 4 * Q + 3
                psA = [ps_A.tile([128, 512], F32, tag="A", name=f"psA{h}_{Q}_{s}") for s in range(2)]
                psD = [ps_small.tile([1, 512], F32, tag="sm", name=f"psD{h}_{Q}_{s}") for s in range(2)]
                for j in range(jmax + 1):
                    off = max(0, 128 * j - q0)
                    w = 512 - off
                    for s_i in range(2):
                        ps_s = ps_sc.tile([128, 512], F32, tag="s")
                        nc.tensor.matmul(
                            ps_s[:, off:],
                            f32r(krot[64 * s_i : 64 * s_i + 64, 128 * j : 128 * j + 128]),
                            f32r(qrot[h][64 * s_i : 64 * s_i + 64, q0 + off : q0 + 512]),
                            start=True,
                            stop=True,
                        )
                        if j >= 4 * Q:
                            nc.vector.tensor_add(
                                ps_s[:, off : off + 128],
                                ps_s[:, off : off + 128],
                                diag_sb[:, j, :],
                            )
                        P = pP.tile([128, 512], F32R, tag="P")
                        nc.scalar.activation(P[:, off:], ps_s[:, off:], AF.Exp)
                        nc.tensor.matmul(
                            psA[s_i][:, off:],
                            f32r(v_sb[:, j, :]),
                            f32r(P[:, off:]),
                            start=(j == 0),
                            stop=(j == jmax),
                        )
                        nc.tensor.matmul(
                            psD[s_i][:, off:],
                            f32r(ones_sb),
                            f32r(P[:, off:]),
                            start=(j == 0),
                            stop=(j == jmax),
                        )

                # denominators -> broadcast reciprocals
                rd = []
                for s_i in range(2):
                    r = psm.tile([1, 512], F32, tag="sm1")
                    nc.vector.reciprocal(r, psD[s_i])
                    rd.append(bcast(r, f"rd{h}_{Q}_{s_i}"))

                ych = y_all[:, h, q0 : q0 + 512]
                t2 = pcb.tile([128, 512], F32, tag="t2c")
                nc.vector.scalar_tensor_tensor(
                    out=t2, in0=psA[1], scalar=lam_sb, in1=rd[1],
                    op0=ALU.mult, op1=ALU.mult,
                )
                t1 = pcb.tile([128, 512], F32, tag="t1c")
                nc.vector.tensor_mul(t1, psA[0], rd[0])
                nc.vector.tensor_sub(ych, t1, t2)

                # LN stats: mu = (1/128) ones . Y ; Esq = (1/128) ones . Y^2
                psMu = ps_small.tile([1, 512], F32, tag="sm")
                nc.tensor.matmul(psMu, f32r(inv_sb), f32r(ych), start=True, stop=True)
                y2 = pcb.tile([128, 512], F32R, tag="y2")
                nc.scalar.activation(y2, ych, AF.Square)
                psSq = ps_small.tile([1, 512], F32, tag="sm")
                nc.tensor.matmul(psSq, f32r(inv_sb), f32r(y2), start=True, stop=True)

                m2 = psm.tile([1, 512], F32, tag="sm1")
                nc.scalar.activation(m2, psMu, AF.Square)
                var = psm.tile([1, 512], F32, tag="sm1")
                nc.vector.scalar_tensor_tensor(
                    out=var, in0=psSq, scalar=float(EPS), in1=m2,
                    op0=ALU.add, op1=ALU.subtract,
                )
                lnv = psm.tile([1, 512], F32, tag="sm1")
                nc.scalar.activation(lnv, var, AF.Ln)
                rstd = psm.tile([1, 512], F32, tag="sm1")
                nc.scalar.activation(rstd, lnv, AF.Exp, scale=-0.5)
                mr = psm.tile([1, 512], F32, tag="sm1")
                nc.vector.tensor_mul(mr, psMu, rstd)

                rstdb = bcast(rstd, f"rstd{h}_{Q}")
                mrb = bcast(mr, f"mr{h}_{Q}")

                # Z = (Y*lnw)*rstdb - mrb*lnw   (in place into y_all)
                tZ = pcb.tile([128, 512], F32, tag="tz")
                nc.vector.scalar_tensor_tensor(
                    out=tZ, in0=ych, scalar=lnw_sb[:, h : h + 1], in1=rstdb,
                    op0=ALU.mult, op1=ALU.mult,
                )
                tU = pcb.tile([128, 512], F32, tag="tu")
                nc.vector.tensor_scalar_mul(tU, mrb, lnw_sb[:, h : h + 1])
                nc.vector.tensor_sub(ych, tZ, tU)



def _attention_v2(tc, nc, qrot, krot, v_sb, diag_sb, lam_sb, lnw_sb,
                  onesm, invm, onesf, y_all, wo, out, daug=False,
                  lam_val=0.0):
    """Attention + interleaved W_O.

    Q-chunk outer / head inner; once all four heads of a Q-chunk have been
    layer-normed into y_all, the W_O matmuls for its four token tiles are
    emitted so they fill TensorE gaps in the next chunk's attention.
    Denominator / LN-stat matmuls use replicated [128,128] stationaries so
    their PSUM outputs need no cross-partition broadcast.  Work tiles share
    one pool tag; PSUM: 4 accumulator banks + 3 score banks + 1 W_O bank.

    daug=True (requires channel-constant lambda + the phase-1 v
    augmentation): the per-j denominator matmuls are dropped; D is the
    channel sum of the finished A accumulator, computed with ONE
    replicated-ones matmul over an f32 SBUF copy of psA (f32, not bf16 —
    the channel sum cancels heavily, so bf16 rounding of A would put ~2%
    noise on D).
    """
    with (
        tc.tile_pool(name="pP", bufs=8 if daug else 12) as pP,
        tc.tile_pool(name="pwk", bufs=16) as pwk,
        tc.tile_pool(name="pac", bufs=2) as pac,
        tc.tile_pool(name="wo_pool", bufs=1) as wo_pool,
        tc.tile_pool(name="ps_acc", bufs=3 if daug else 4, space="PSUM") as ps_acc,
        tc.tile_pool(name="ps_sc", bufs=2 if daug else 3, space="PSUM") as ps_sc,
        tc.tile_pool(name="ps_wo", bufs=1, space="PSUM") as ps_wo,
    ):
        wo_sb = wo_pool.tile([128, NH, D], BF16)
        nc.sync.dma_start(out=wo_sb, in_=wo[:, :].rearrange("(n p) c -> p n c", p=128))

        def wk(nm):
            return pwk.tile([128, 512], F32, tag="wk", name=nm)

        # big-Q chunks are PE-heavy (long j-loops), small-Q chunks are
        # DVE-heavy (fixed combine/LN tail); order them so tails overlap
        # long j-loops.  W_O for a Q-chunk fires once its heads complete.
        for Q in (3, 0, 2, 1):
            q0 = 512 * Q
            jmax = 4 * Q + 3
            for h in range(NH):
                t1 = None
                ych = y_all[:, h, q0 : q0 + 512]
                if daug:
                    # fused j-loop: both split-halves share one [128,1024]
                    # score tile so exp runs as one wide activation (ACT
                    # per-instruction overhead is ~293 ns — halving the
                    # instruction count matters more than tile shape).
                    psA2 = [ps_acc.tile([128, 512], F32, tag="acc",
                                        name=f"psA{h}_{Q}_{s}")
                            for s in range(2)]
                    for j in range(jmax + 1):
                        off = max(0, 128 * j - q0)
                        ps_s = ps_sc.tile([128, 1024], F32, tag="s",
                                          name=f"s{h}_{Q}_{j}")
                        for s_i in range(2):
                            nc.tensor.matmul(
                                ps_s[:, 512 * s_i + off : 512 * s_i + 512],
                                krot[64 * s_i : 64 * s_i + 64,
                                     128 * j : 128 * j + 128],
                                qrot[h][64 * s_i : 64 * s_i + 64,
                                        q0 + off : q0 + 512],
                                start=True,
                                stop=True,
                            )
                        P = pP.tile([128, 1024], BF16, tag="P",
                                    name=f"P{h}_{Q}_{j}")
                        if off == 0:
                            nc.scalar.activation(P, ps_s, AF.Exp)
                        else:
                            nc.scalar.activation(
                                P[:, off:512], ps_s[:, off:512], AF.Exp)
                            nc.scalar.activation(
                                P[:, 512 + off :], ps_s[:, 512 + off :],
                                AF.Exp)
                        if j >= 4 * Q:
                            for s_i in range(2):
                                nc.vector.tensor_mul(
                                    P[:, 512 * s_i + off :
                                      512 * s_i + off + 128],
                                    P[:, 512 * s_i + off :
                                      512 * s_i + off + 128],
                                    diag_sb[:, j, :],
                                )
                        for s_i in range(2):
                            nc.tensor.matmul(
                                psA2[s_i][:, off:], v_sb[:, j, :],
                                P[:, 512 * s_i + off : 512 * s_i + 512],
                                start=(j == 0), stop=(j == jmax),
                            )
                    for s_i in range(2):
                        rd = wk(f"rd{h}_{Q}_{s_i}")
                        ac = pac.tile([128, 512], F32R, tag="ac",
                                      name=f"ac{h}_{Q}_{s_i}")
                        nc.vector.tensor_copy(ac, psA2[s_i])
                        psDs = ps_acc.tile([128, 512], F32, tag="acc",
                                           name=f"psDs{h}_{Q}_{s_i}")
                        nc.tensor.matmul(psDs, onesf, ac, start=True,
                                         stop=True)
                        # 1/D as exp(-ln(D)) on ACT: nc.vector.reciprocal is
                        # an 8-cycle/elem iterative divide (~4.3us per tile);
                        # Ln+Exp share the already-loaded table set and cost
                        # ~0.7us each.
                        lnD = wk(f"lnD{h}_{Q}_{s_i}")
                        nc.scalar.activation(lnD, psDs, AF.Ln)
                        nc.scalar.activation(rd, lnD, AF.Exp, scale=-1.0)
                        if s_i == 0:
                            t1 = wk(f"t1_{h}_{Q}")
                            nc.vector.tensor_mul(t1, ac, rd)
                        else:
                            t2 = wk(f"t2_{h}_{Q}")
                            nc.vector.scalar_tensor_tensor(
                                out=t2, in0=ac, scalar=lam_sb, in1=rd,
                                op0=ALU.mult, op1=ALU.mult,
                            )
                            nc.vector.tensor_sub(ych, t1, t2)
                else:
                  for s_i in range(2):
                    psA = ps_acc.tile([128, 512], F32, tag="acc",
                                      name=f"psA{h}_{Q}_{s_i}")
                    psD = ps_acc.tile([128, 512], F32, tag="acc",
                                      name=f"psD{h}_{Q}_{s_i}")
                    for j in range(jmax + 1):
                        off = max(0, 128 * j - q0)
                        ps_s = ps_sc.tile([128, 512], F32, tag="s",
                                          name=f"s{h}_{Q}_{j}_{s_i}")
                        nc.tensor.matmul(
                            ps_s[:, off:],
                            krot[64 * s_i : 64 * s_i + 64,
                                 128 * j : 128 * j + 128],
                            qrot[h][64 * s_i : 64 * s_i + 64,
                                    q0 + off : q0 + 512],
                            start=True,
                            stop=True,
                        )
                        P = pP.tile([128, 512], BF16, tag="P",
                                    name=f"P{h}_{Q}_{j}_{s_i}")
                        nc.scalar.activation(P[:, off:], ps_s[:, off:], AF.Exp)
                        if j >= 4 * Q:
                            # 0/1 causal mask applied post-exp (SBUF, 2x mode,
                            # off the score-PSUM critical path); exp(s)*0 is
                            # bit-identical to exp(s - 1e9)
                            nc.vector.tensor_mul(
                                P[:, off : off + 128],
                                P[:, off : off + 128],
                                diag_sb[:, j, :],
                            )
                        nc.tensor.matmul(
                            psA[:, off:], v_sb[:, j, :], P[:, off:],
                            start=(j == 0), stop=(j == jmax),
                        )
                        nc.tensor.matmul(
                            psD[:, off:], onesm, P[:, off:],
                            start=(j == 0), stop=(j == jmax),
                        )
                    rd = wk(f"rd{h}_{Q}_{s_i}")
                    nc.vector.reciprocal(rd, psD)
                    if s_i == 0:
                        t1 = wk(f"t1_{h}_{Q}")
                        nc.vector.tensor_mul(t1, psA, rd)
                    else:
                        t2 = wk(f"t2_{h}_{Q}")
                        nc.vector.scalar_tensor_tensor(
                            out=t2, in0=psA, scalar=lam_sb, in1=rd,
                            op0=ALU.mult, op1=ALU.mult,
                        )
                        nc.vector.tensor_sub(ych, t1, t2)

                # LN for this (h, Q) chunk, replicated stats.
                y2 = pwk.tile([128, 512], BF16, tag="y2b", name=f"y2_{h}_{Q}",
                              bufs=2)
                nc.vector.tensor_mul(y2, ych, ych)
                psSq = ps_acc.tile([128, 512], F32, tag="acc",
                                   name=f"psSq{h}_{Q}")
                nc.tensor.matmul(psSq, invm, y2, start=True, stop=True)
                var = wk(f"var{h}_{Q}")
                if daug:
                    # daug makes the channel mean a compile-time constant:
                    # sum_ch A'_s = D_s by construction, so sum_ch ych =
                    # 1 - lam exactly and mu = (1-lam)/128 — the psMu
                    # matmul / staging copy / m2 all drop out.
                    mu_c = (1.0 - lam_val) / 128.0
                    nc.vector.tensor_scalar(
                        out=var, in0=psSq,
                        scalar1=float(EPS - mu_c * mu_c), scalar2=None,
                        op0=ALU.add,
                    )
                else:
                    psMu = ps_acc.tile([128, 512], F32, tag="acc",
                                       name=f"psMu{h}_{Q}")
                    nc.tensor.matmul(psMu, invm, ych, start=True, stop=True)
                    musb = wk(f"musb{h}_{Q}")
                    nc.vector.tensor_copy(musb, psMu)
                    m2 = wk(f"m2_{h}_{Q}")
                    nc.vector.tensor_mul(m2, musb, musb)
                    nc.vector.scalar_tensor_tensor(
                        out=var, in0=psSq, scalar=float(EPS), in1=m2,
                        op0=ALU.add, op1=ALU.subtract,
                    )
                lnv = wk(f"lnv{h}_{Q}")
                nc.scalar.activation(lnv, var, AF.Ln)
                rstd = wk(f"rstd{h}_{Q}")
                nc.scalar.activation(rstd, lnv, AF.Exp, scale=-0.5)
                tc1 = wk(f"tc1_{h}_{Q}")
                if daug:
                    nc.vector.tensor_scalar(
                        out=tc1, in0=ych, scalar1=float(mu_c), scalar2=None,
                        op0=ALU.subtract,
                    )
                else:
                    nc.vector.tensor_sub(tc1, ych, musb)
                nc.vector.scalar_tensor_tensor(
                    out=ych, in0=tc1, scalar=lnw_sb[:, h : h + 1], in1=rstd,
                    op0=ALU.mult, op1=ALU.mult,
                )

            # W_O for this Q-chunk's four token tiles.  For the final
            # chunk in the order there is no attention left to fill PE
            # gaps, so its groups also borrow the (now idle) score banks.
            for u in range(4):
                tt = 4 * Q + u
                for cci in range(4):
                    if Q == 1 and cci % 2 == 1:
                        if daug:
                            psO = ps_sc.tile([128, 1024], F32, tag="s",
                                             name=f"psO{tt}_{cci}")[:, 0:512]
                        else:
                            psO = ps_sc.tile([128, 512], F32, tag="s",
                                             name=f"psO{tt}_{cci}")
                    else:
                        psO = ps_wo.tile([128, 512], F32, tag="O",
                                         name=f"psO{tt}_{cci}")
                    for h in range(NH):
                        nc.tensor.matmul(
                            psO,
                            y_all[:, h, 128 * tt : 128 * tt + 128],
                            wo_sb[:, h, 512 * cci : 512 * cci + 512],
                            start=(h == 0),
                            stop=(h == NH - 1),
                        )
                    osb = pwk.tile([128, 512], F32, tag="osb",
                                   name=f"osb{tt}_{cci}", bufs=5)
                    if daug:
                        # ACT carries the exp+recip load in the daug build;
                        # keep all output staging on the DVE.
                        nc.vector.tensor_copy(osb, psO)
                    elif (tt + cci) % 2 == 0:
                        nc.scalar.copy(osb, psO)
                    else:
                        nc.vector.tensor_copy(osb, psO)
                    nc.sync.dma_start(
                        out=out[128 * tt : 128 * tt + 128,
                                512 * cci : 512 * cci + 512],
                        in_=osb,
                    )


def _attention_daug(tc, nc, qrot, krot, v_sb, diag_sb, lam_sb, lnw_sb,
                    ones_sb, y_all, lam_val):
    """D-free attention: with v' columns summing to 1, D_s = sum_ch A'_s.
    Valid when the lambda vector is channel-constant (per-head LN then
    cancels the channel-constant shift t_s introduced by the augmentation)."""
    mu_const = (1.0 - lam_val) / 128.0
    with (
        tc.tile_pool(name="pP", bufs=3) as pP,
        tc.tile_pool(name="pbc", bufs=4) as pbc,
        tc.tile_pool(name="pcb", bufs=2) as pcb,
        tc.tile_pool(name="psm", bufs=4) as psm,
        tc.tile_pool(name="pstat", bufs=2) as pstat,
        tc.tile_pool(name="pdram", bufs=6, space="DRAM") as pdram,
        tc.tile_pool(name="ps_A", bufs=4, space="PSUM") as ps_A,
        tc.tile_pool(name="ps_sc", bufs=3, space="PSUM") as ps_sc,
    ):
        def bcast(src, nm, width):
            scr = pdram.tile([1, width], F32, tag="scr", name=f"scr_{nm}")
            nc.sync.dma_start(out=scr, in_=src)
            dst = pbc.tile([128, width], F32, tag=f"bc{width}", name=f"bc_{nm}",
                           bufs=4 if width == 512 else 2)
            nc.gpsimd.dma_start(out=dst, in_=scr.to_broadcast([128, width]))
            return dst

        for h in range(NH):
            sqsb = pstat.tile([1, S], F32, tag="sqrow", name=f"sq{h}", bufs=1)
            for Q in range(NCT):
                q0 = 512 * Q
                jmax = 4 * Q + 3
                psA = [ps_A.tile([128, 512], F32, tag="A", name=f"psA{h}_{Q}_{s}")
                       for s in range(2)]
                for j in range(jmax + 1):
                    off = max(0, 128 * j - q0)
                    ps_s = ps_sc.tile([128, 1024], F32, tag="s")
                    for s_i in range(2):
                        nc.tensor.matmul(
                            ps_s[:, 512 * s_i + off : 512 * s_i + 512],
                            krot[64 * s_i : 64 * s_i + 64,
                                 128 * j : 128 * j + 128],
                            qrot[h][64 * s_i : 64 * s_i + 64,
                                    q0 + off : q0 + 512],
                            start=True,
                            stop=True,
                        )
                        if j >= 4 * Q:
                            nc.vector.tensor_add(
                                ps_s[:, 512 * s_i + off : 512 * s_i + off + 128],
                                ps_s[:, 512 * s_i + off : 512 * s_i + off + 128],
                                diag_sb[:, j, :],
                            )
                    P = pP.tile([128, 1024], F32R, tag="P")
                    if off == 0:
                        nc.scalar.activation(P, ps_s, AF.Exp)
                    else:
                        nc.scalar.activation(
                            P[:, off:512], ps_s[:, off:512], AF.Exp)
                        nc.scalar.activation(
                            P[:, 512 + off :], ps_s[:, 512 + off :], AF.Exp)
                    for s_i in range(2):
                        nc.tensor.matmul(
                            psA[s_i][:, off:],
                            v_sb[:, j, :],
                            P[:, 512 * s_i + off : 512 * s_i + 512],
                            start=(j == 0),
                            stop=(j == jmax),
                        )

                # denominators = channel sums of A'; combine; LN row stats
                ac, rdb = [], []
                for s_i in range(2):
                    c = pcb.tile([128, 512], F32R, tag=f"ac{s_i}",
                                 name=f"ac{s_i}_{h}_{Q}", bufs=2)
                    nc.vector.tensor_copy(c, psA[s_i])
                    ac.append(c)
                for s_i in range(2):
                    Ssum = ps_A.tile([1, 512], F32, tag="A",
                                     name=f"S{s_i}_{h}_{Q}")
                    nc.tensor.matmul(Ssum, ones_sb, ac[s_i], start=True, stop=True)
                    r = psm.tile([1, 512], F32, tag="sm1", name=f"rd{s_i}_{h}_{Q}")
                    nc.vector.reciprocal(r, Ssum)
                    rdb.append(bcast(r, f"rd{s_i}_{h}_{Q}", 512))

                ych = y_all[:, h, q0 : q0 + 512]
                t2 = pcb.tile([128, 512], F32, tag="t2c")
                nc.vector.scalar_tensor_tensor(
                    out=t2, in0=ac[1], scalar=lam_sb, in1=rdb[1],
                    op0=ALU.mult, op1=ALU.mult,
                )
                t1 = pcb.tile([128, 512], F32, tag="t1c")
                nc.vector.tensor_mul(t1, ac[0], rdb[0])
                nc.vector.tensor_sub(ych, t1, t2)

                y2 = pcb.tile([128, 512], F32R, tag="y2")
                nc.vector.tensor_mul(y2, ych, ych)
                psSq = ps_A.tile([1, 512], F32, tag="A", name=f"psSq{h}_{Q}")
                nc.tensor.matmul(psSq, ones_sb, y2, start=True, stop=True)
                nc.vector.tensor_scalar(
                    out=sqsb[:, q0 : q0 + 512], in0=psSq,
                    scalar1=1.0 / 128.0, scalar2=None, op0=ALU.mult,
                )

            # per-head LN tail: var = E[y^2] - mu^2 (mu = (1-lam)/128 exact)
            var = psm.tile([1, S], F32, tag="var", name=f"var{h}", bufs=2)
            nc.vector.tensor_scalar(
                out=var, in0=sqsb,
                scalar1=float(EPS - mu_const * mu_const), scalar2=None,
                op0=ALU.add,
            )
            lnv = psm.tile([1, S], F32, tag="var", name=f"lnv{h}", bufs=2)
            nc.scalar.activation(lnv, var, AF.Ln)
            rstd = psm.tile([1, S], F32, tag="var", name=f"rstd{h}", bufs=2)
            nc.scalar.activation(rstd, lnv, AF.Exp, scale=-0.5)

            for Q in range(NCT):
                q0 = 512 * Q
                rb = bcast(rstd[:, q0 : q0 + 512], f"rstd{h}_{Q}", 512)
                ych = y_all[:, h, q0 : q0 + 512]
                tZ = pcb.tile([128, 512], F32, tag="tz", bufs=2)
                nc.vector.scalar_tensor_tensor(
                    out=tZ, in0=ych, scalar=lnw_sb[:, h : h + 1], in1=rb,
                    op0=ALU.mult, op1=ALU.mult,
                )
                tU = pcb.tile([128, 512], F32, tag="tu", bufs=2)
                nc.vector.tensor_scalar(
                    out=tU, in0=rb, scalar1=lnw_sb[:, h : h + 1],
                    scalar2=float(mu_const), op0=ALU.mult, op1=ALU.mult,
                )
                nc.vector.tensor_sub(ych, tZ, tU)

def _wo_phase(tc, nc, f32r, y_all, wo, out):
    with (
        tc.tile_pool(name="ps_wo", bufs=2, space="PSUM") as ps_wo,
        tc.tile_pool(name="ostage", bufs=4) as ostage,
        tc.tile_pool(name="wo_pool", bufs=1) as wo_pool,
    ):
        wo_sb = wo_pool.tile([128, NH, D], BF16)
        nc.sync.dma_start(out=wo_sb, in_=wo[:, :].rearrange("(n p) c -> p n c", p=128))
        for tt in range(NT):
            psO = ps_wo.tile([128, 2048], F32, tag="O")
            for cci in range(4):
                for h in range(NH):
                    nc.tensor.matmul(
                        psO[:, 512 * cci : 512 * cci + 512],
                        f32r(y_all[:, h, 128 * tt : 128 * tt + 128]),
                        f32r(wo_sb[:, h, 512 * cci : 512 * cci + 512]),
                        start=(h == 0),
                        stop=(h == NH - 1),
                    )
            osb = ostage.tile([128, 2048], F32, tag="osb")
            nc.scalar.copy(osb, psO)
            nc.sync.dma_start(
                out=out[128 * tt : 128 * tt + 128, :], in_=osb,
            )


# --------------------------------------------------------------------------
# host side
# --------------------------------------------------------------------------

_PERM = np.concatenate([np.arange(0, 64, 2), np.arange(1, 64, 2)])


def prep_core_inputs(x, mask, freq_cos, freq_sin, Wq, Wkv, Wo,
                     lambda_q1, lambda_k1, lambda_q2, lambda_k2, ln_weight,
                     b, g):
    lam_init = np.float32(0.8) - np.float32(0.6) * np.exp(
        np.float32(-0.3 * LAYER_IDX)
    )
    scale = np.float32(HD ** -0.5)

    Wq4 = Wq.reshape(D, H, 2, HD)[:, 4 * g : 4 * g + 4]
    Wq4 = Wq4[..., _PERM] * scale
    wq = np.ascontiguousarray(Wq4.reshape(D, CHQ)).astype(NPBF16)

    Wkv5 = Wkv.reshape(D, 2, HKV, 2 * HD)
    wk = Wkv5[:, 0, g].reshape(D, 2, HD)[..., _PERM].reshape(D, 2 * HD)
    wk = np.ascontiguousarray(wk).astype(NPBF16)
    wv = np.ascontiguousarray(Wkv5[:, 1, g]).astype(NPBF16)

    cosT = freq_cos.T.astype(np.float32)   # [32, S]
    sinT = freq_sin.T.astype(np.float32)
    cc = np.ascontiguousarray(np.vstack([cosT, cosT, cosT, cosT])).astype(NPBF16)
    sp = np.ascontiguousarray(np.vstack([-sinT, sinT, -sinT, sinT])).astype(NPBF16)

    m = mask[0, 0]
    diagmT = np.ascontiguousarray(
        np.stack(
            [(m[i * 128 : (i + 1) * 128, i * 128 : (i + 1) * 128].T
              > NEG_THRESH).astype(np.float32) for i in range(NT)]
        ),
    ).astype(NPBF16)

    lamvec = (
        np.exp(lambda_q1 * lambda_k1) - np.exp(lambda_q2 * lambda_k2) + lam_init
    ).astype(np.float32).reshape(128, 1)
    lnw = np.ascontiguousarray(
        (ln_weight[4 * g : 4 * g + 4] * (np.float32(1.0) - lam_init)).T,
        dtype=np.float32,
    )
    wo = np.ascontiguousarray(Wo[CHQ * g : CHQ * (g + 1)]).astype(NPBF16)
    xT = np.ascontiguousarray(x[b].T).astype(NPBF16)

    return {
        "xT": xT, "wq": wq, "wk": wk, "wv": wv, "cc": cc, "sp": sp,
        "diagmT": diagmT, "lam": lamvec, "lnw": lnw, "wo": wo,
        "vecs": np.ascontiguousarray(
            np.stack([np.ones(128), np.full(128, 1.0 / 128.0)], axis=1),
            dtype=np.float32),
        "mats": np.ascontiguousarray(np.concatenate(
            [np.ones((128, 128)), np.full((128, 128), 1.0 / 128.0)], axis=1)
        ).astype(NPBF16),
        "matsf": np.ones((128, 128), dtype=np.float32),
        "swm": _swap_matrix(),
    }


def _swap_matrix():
    # permutation swapping the A/B 32-row halves of each 64-row block
    sw = np.zeros((128, 128), dtype=np.float32)
    for f in range(2):
        for i in range(32):
            sw[64 * f + i, 64 * f + 32 + i] = 1.0
            sw[64 * f + 32 + i, 64 * f + i] = 1.0
    return sw.astype(NPBF16)


def check_mask_causal(mask):
    m = mask[0, 0]
    lower_ok = True
    upper_ok = True
    tril = np.tril_indices(S)
    if not np.all(m[tril] == 0.0):
        lower_ok = False
    triu = np.triu_indices(S, k=1)
    if not np.all(m[triu] <= NEG_THRESH):
        upper_ok = False
    return lower_ok and upper_ok


_NC_CACHE = {}


def _lam_vec(inputs):
    lam_init = np.float32(0.8) - np.float32(0.6) * np.exp(np.float32(-0.3 * LAYER_IDX))
    return (
        np.exp(inputs["lambda_q1"] * inputs["lambda_k1"])
        - np.exp(inputs["lambda_q2"] * inputs["lambda_k2"])
        + lam_init
    ).astype(np.float32)


def _get_nc(daug=False, lam_val=0.0):
    key = (bool(daug), float(lam_val) if daug else 0.0)
    if key not in _NC_CACHE:
        _NC_CACHE[key] = build_nc(daug, lam_val)
    return _NC_CACHE[key]


def kernel(**inputs) -> np.ndarray:
    inputs = {k: np.asarray(v) for k, v in inputs.items()}
    assert check_mask_causal(inputs["mask"]), (
        "kernel assumes a causal additive mask (0 lower / <=-1e8 strict upper); "
        "got something else"
    )
    # lambda channel-constant (true for the reference's all-ones lambda
    # vectors) enables the denominator-free daug path; anything else falls
    # back to explicit denominator accumulation.
    lamvec = _lam_vec(inputs)
    daug = bool(np.ptp(lamvec) == 0.0)
    nc = _get_nc(daug, float(lamvec[0]) if daug else 0.0)
    in_maps = []
    for b in range(B):
        for g in range(HKV):
            in_maps.append(prep_core_inputs(
                inputs["x"], inputs["mask"], inputs["freq_cos"], inputs["freq_sin"],
                inputs["Wq"], inputs["Wkv"], inputs["Wo"],
                inputs["lambda_q1"], inputs["lambda_k1"],
                inputs["lambda_q2"], inputs["lambda_k2"], inputs["ln_weight"],
                b, g,
            ))
    res = run_bass_kernel_spmd(nc, in_maps, list(range(B * HKV)))
    parts = [r["out"] for r in res.results]
    out = np.stack([
        np.sum(np.stack(parts[0:HKV]), axis=0, dtype=np.float32),
        np.sum(np.stack(parts[HKV : 2 * HKV]), axis=0, dtype=np.float32),
    ])
    return out.astype(np.float32)

